# revision 17
# baseline (speedup 1.0000x reference)
"""Trainium2 Bass kernel for a GNN message-passing layer.

Reference computation (per batch b):
    m   = relu(h @ W1.T + b1)
    m   = relu(m @ W2.T + b2)
    msg = relu(A @ m)
    gx  = msg @ W_ih.T + b_ih ; gh = h @ W_hh.T + b_hh   (gates r,z,n)
    r = sig(gxr+ghr); z = sig(gxz+ghz); n = tanh(gxn + r*ghn)
    out = (1-z)*n + z*h

Sharding: pure data-parallel over B (B == n_cores == 8, one batch per
NeuronCore, no collectives). Host pre-transposes per-batch tensors into
feature-major layout so the dominant tensor A streams through the PE in
its natural layout (contraction dim on partitions) with zero on-device
transposes.

Per-core device program (all feature-major, [feature(part) x nodes]):
    m1T  = relu(W1 @ hT + b1)                  4 matmuls  (f32r, N=512)
    m2n  = node-major m2 blocks                16 matmuls (f32r, N=128)
    for each 512-node chunk c:
        msgT_c = relu(sum_k m2n_k.T @ AT[k, c])  16 matmuls (f32r, N=512)
        gates: 6 matmuls in plain fp32 (precision: sigmoid/tanh preacts
        are ~+-400 here, so operand rounding must stay at fp32 level)
        GRU elementwise on DVE/ACT
"""

import numpy as np

B, N, H = 8, 2048, 128
NCHUNK = 512
NCH = N // NCHUNK  # 4
KBLK = N // 128    # 16

_CACHE = {}


def _build_program():
    import concourse.bacc as bacc
    import concourse.tile as tile
    import concourse.mybir as mybir
    from concourse.alu_op_type import AluOpType

    f32 = mybir.dt.float32
    f32r = mybir.dt.float32r
    ACT = mybir.ActivationFunctionType

    nc = bacc.Bacc("TRN2", target_bir_lowering=False, debug=False, num_devices=B)

    # ---- DRAM I/O (per-core shard, host-pre-transposed) ----
    hT_d = nc.dram_tensor("hT", [H, N], f32, kind="ExternalInput").ap()
    # A2[q, s] = one contiguous [128, 2048] slab: 4 k-blocks (t=0..3, k=4s+t)
    # of A^T columns for node-chunk q. Host packs it this way so every DMA
    # is a full-rate 1MB contiguous transfer AND chunk q's accumulation
    # completes after quarter q of the stream.
    A2_d = nc.dram_tensor("A2", [NCH, KBLK // 4, H, N], f32r, kind="ExternalInput").ap()
    w1t_d = nc.dram_tensor("W1T", [H, H], f32, kind="ExternalInput").ap()
    w2t_d = nc.dram_tensor("W2T", [H, H], f32, kind="ExternalInput").ap()
    wih_d = nc.dram_tensor("WihT", [H, 3 * H], f32, kind="ExternalInput").ap()
    whh_d = nc.dram_tensor("WhhT", [H, 3 * H], f32r, kind="ExternalInput").ap()
    b1_d = nc.dram_tensor("b1c", [H, 1], f32, kind="ExternalInput").ap()
    b2b_d = nc.dram_tensor("b2b", [H, H], f32, kind="ExternalInput").ap()
    brz_d = nc.dram_tensor("brz", [H, 2], f32, kind="ExternalInput").ap()
    bihn_d = nc.dram_tensor("bihn", [H, 1], f32, kind="ExternalInput").ap()
    bhhn_d = nc.dram_tensor("bhhn", [H, 1], f32, kind="ExternalInput").ap()
    out_d = nc.dram_tensor("outT", [H, N], f32, kind="ExternalOutput").ap()

    with tile.TileContext(nc) as tc:
        with (
            tc.tile_pool(name="consts", bufs=1) as cp,
            tc.tile_pool(name="big", bufs=1) as bp,
            tc.tile_pool(name="at", bufs=5) as ap_,
            tc.tile_pool(name="msgp", bufs=2) as mp,
            tc.tile_pool(name="tmp", bufs=2) as tp,
            tc.tile_pool(name="outp", bufs=2) as op_,
            tc.tile_pool(name="psum", bufs=1, space="PSUM") as pp,
        ):
            # ---- constant loads ----
            w1t = cp.tile([H, H], f32, tag="w1t")
            w2t = cp.tile([H, H], f32, tag="w2t")
            wih = cp.tile([H, 3 * H], f32, tag="wih")
            whh = cp.tile([H, 3 * H], f32r, tag="whh")
            b1 = cp.tile([H, 1], f32, tag="b1")
            b2b = cp.tile([H, H], f32, tag="b2b")
            brz = cp.tile([H, 2], f32, tag="brz")
            bihn = cp.tile([H, 1], f32, tag="bihn")
            bhhn = cp.tile([H, 1], f32, tag="bhhn")
            hT = bp.tile([H, N], f32, tag="hT")
            hTr = bp.tile([H, N], f32r, tag="hTr")  # f32r copy for gh matmuls
            m1T = bp.tile([H, N], f32, tag="m1T")
            m2n = bp.tile([H, N], f32r, tag="m2n")  # block k = cols [128k,128k+128)

            # constants + hT on the ACT (scalar) HWDGE ring so the sync ring
            # can start streaming A at t=0
            nc.scalar.dma_start(hT[:], hT_d[:])
            nc.scalar.dma_start(w1t[:], w1t_d[:])
            nc.scalar.dma_start(w2t[:], w2t_d[:])
            nc.scalar.dma_start(wih[:], wih_d[:])
            nc.scalar.dma_start(whh[:], whh_d[:])
            nc.scalar.dma_start(b1[:], b1_d[:])
            nc.scalar.dma_start(b2b[:], b2b_d[:])
            nc.scalar.dma_start(brz[:], brz_d[:])
            nc.scalar.dma_start(bihn[:], bihn_d[:])
            nc.scalar.dma_start(bhhn[:], bhhn_d[:])

            nc.scalar.copy(hTr[:], hT[:])

            # ---- m1T = relu(W1 @ hT + b1), feature-major ----
            for c in range(NCH):
                sl = slice(c * NCHUNK, (c + 1) * NCHUNK)
                ps_m1 = pp.tile([H, NCHUNK], f32, tag="acc", bufs=4)
                nc.tensor.matmul(
                    ps_m1[:], w1t[:], hT[:, sl],
                    start=True, stop=True,
                )
                nc.scalar.activation(m1T[:, sl], ps_m1[:], ACT.Relu, bias=b1[:, 0:1])

            # ---- m2 node-major blocks: m2n[:, blk k] = relu(m1T_k.T @ W2T + b2) ----
            for k in range(KBLK):
                kb = slice(k * H, (k + 1) * H)
                ps_m2 = pp.tile([H, H], f32, tag="acc", bufs=4)
                nc.tensor.matmul(
                    ps_m2[:], m1T[:, kb], w2t[:],
                    start=True, stop=True,
                )
                m2pre = tp.tile([H, H], f32, tag="m2pre")
                nc.vector.tensor_add(m2pre[:], ps_m2[:], b2b[:])
                nc.scalar.activation(m2n[:, kb], m2pre[:], ACT.Relu)

            # ---- stream A quarter by quarter; chunk q's msg completes after
            # quarter q, so its gates overlap quarter q+1's DMA ----
            for q in range(NCH):
                sl = slice(q * NCHUNK, (q + 1) * NCHUNK)

                ps_msg = pp.tile([H, NCHUNK], f32, tag="msg", bufs=3)
                for s in range(KBLK // 4):
                    at = ap_.tile([H, N], f32r, tag="at")
                    nc.sync.dma_start(at[:], A2_d[q, s])
                    for t in range(4):
                        k = 4 * s + t
                        nc.tensor.matmul(
                            ps_msg[:],
                            m2n[:, k * H:(k + 1) * H],
                            at[:, t * NCHUNK:(t + 1) * NCHUNK],
                            start=(k == 0), stop=(k == KBLK - 1),
                        )

                # gh gates for this chunk (depend only on h): emitted here so
                # the PE runs them in this quarter's DMA gaps
                ghr = tp.tile([H, NCHUNK], f32, tag="ghr")
                ps_ghr = pp.tile([H, NCHUNK], f32, tag="acc", bufs=4)
                nc.tensor.matmul(ps_ghr[:], whh[:, 0:H], hTr[:, sl], start=True, stop=True)
                nc.scalar.activation(ghr[:], ps_ghr[:], ACT.Identity, bias=brz[:, 0:1])
                ghz = tp.tile([H, NCHUNK], f32, tag="ghz")
                ps_ghz = pp.tile([H, NCHUNK], f32, tag="acc", bufs=4)
                nc.tensor.matmul(ps_ghz[:], whh[:, H:2 * H], hTr[:, sl], start=True, stop=True)
                nc.scalar.activation(ghz[:], ps_ghz[:], ACT.Identity, bias=brz[:, 1:2])
                ghn = tp.tile([H, NCHUNK], f32, tag="ghn")
                ps_ghn = pp.tile([H, NCHUNK], f32, tag="acc", bufs=4)
                nc.tensor.matmul(ps_ghn[:], whh[:, 2 * H:3 * H], hTr[:, sl], start=True, stop=True)
                nc.scalar.activation(ghn[:], ps_ghn[:], ACT.Identity, bias=bhhn[:, 0:1])

                msgT = mp.tile([H, NCHUNK], f32, tag="msgT")
                nc.scalar.activation(msgT[:], ps_msg[:], ACT.Relu)

                ps_gxr = pp.tile([H, NCHUNK], f32, tag="acc", bufs=4)
                nc.tensor.matmul(ps_gxr[:], wih[:, 0:H], msgT[:], start=True, stop=True)
                rpre = tp.tile([H, NCHUNK], f32, tag="rpre")
                nc.vector.tensor_add(rpre[:], ps_gxr[:], ghr[:])
                r = tp.tile([H, NCHUNK], f32, tag="r")
                nc.scalar.activation(r[:], rpre[:], ACT.Sigmoid)

                ps_gxz = pp.tile([H, NCHUNK], f32, tag="acc", bufs=4)
                nc.tensor.matmul(ps_gxz[:], wih[:, H:2 * H], msgT[:], start=True, stop=True)
                zpre = tp.tile([H, NCHUNK], f32, tag="zpre")
                nc.vector.tensor_add(zpre[:], ps_gxz[:], ghz[:])
                z = tp.tile([H, NCHUNK], f32, tag="z")
                nc.scalar.activation(z[:], zpre[:], ACT.Sigmoid)

                ps_gxn = pp.tile([H, NCHUNK], f32, tag="acc", bufs=4)
                nc.tensor.matmul(ps_gxn[:], wih[:, 2 * H:3 * H], msgT[:], start=True, stop=True)

                # t = r * (ghn + bhhn) ; npre = t + gxn ; n = tanh(npre + bihn)
                t = tp.tile([H, NCHUNK], f32, tag="t")
                nc.vector.tensor_mul(t[:], r[:], ghn[:])
                npre = tp.tile([H, NCHUNK], f32, tag="npre")
                nc.vector.tensor_add(npre[:], t[:], ps_gxn[:])
                nn = tp.tile([H, NCHUNK], f32, tag="nn")
                nc.scalar.activation(nn[:], npre[:], ACT.Tanh, bias=bihn[:, 0:1])

                # out = n + z * (h - n); chunks 0-2 on otherwise-idle GPSIMD
                # (overlaps next quarter's stream), last chunk on fast DVE
                # since it is the exposed tail
                eng = nc.vector if q == NCH - 1 else nc.gpsimd
                d = tp.tile([H, NCHUNK], f32, tag="d")
                eng.tensor_sub(d[:], hT[:, sl], nn[:])
                e = tp.tile([H, NCHUNK], f32, tag="e")
                eng.tensor_mul(e[:], z[:], d[:])
                outc = op_.tile([H, NCHUNK], f32, tag="outc")
                eng.tensor_add(outc[:], nn[:], e[:])
                nc.scalar.dma_start(out_d[:, sl], outc[:])

    nc.compile()
    return nc


def _get_program():
    if "nc" not in _CACHE:
        _CACHE["nc"] = _build_program()
    return _CACHE["nc"]


def _make_in_maps(h, A, W1, b1, W2, b2, W_ih, W_hh, b_ih, b_hh):
    f = np.float32
    shared = {
        "W1T": np.ascontiguousarray(W1.T, dtype=f),
        "W2T": np.ascontiguousarray(W2.T, dtype=f),
        "WihT": np.ascontiguousarray(W_ih.T, dtype=f),
        "WhhT": np.ascontiguousarray(W_hh.T, dtype=f),
        "b1c": np.ascontiguousarray(b1.reshape(H, 1), dtype=f),
        "b2b": np.ascontiguousarray(np.tile(b2.reshape(1, H), (H, 1)), dtype=f),
        "brz": np.ascontiguousarray(
            np.stack([(b_ih + b_hh)[0:H], (b_ih + b_hh)[H:2 * H]], axis=1), dtype=f
        ),
        "bihn": np.ascontiguousarray(b_ih[2 * H:3 * H].reshape(H, 1), dtype=f),
        "bhhn": np.ascontiguousarray(b_hh[2 * H:3 * H].reshape(H, 1), dtype=f),
    }
    in_maps = []
    for b in range(B):
        m = dict(shared)
        m["hT"] = np.ascontiguousarray(np.asarray(h[b]).T, dtype=f)
        # A2[q, s, p, t*512+j] = A^T[(4s+t)*128 + p, q*512 + j]
        AT = np.asarray(A[b]).T.astype(f)                      # [2048 m, 2048 n]
        A2 = (AT.reshape(KBLK // 4, 4, H, NCH, NCHUNK)         # [s, t, p, q, j]
                .transpose(3, 0, 2, 1, 4)                      # [q, s, p, t, j]
                .reshape(NCH, KBLK // 4, H, N))
        m["A2"] = np.ascontiguousarray(A2)
        in_maps.append(m)
    return in_maps


def run(inputs, trace=False, trace_cores=None):
    """Build (cached), run on 8 cores, return (output, BassKernelResults)."""
    from concourse.bass_utils import run_bass_kernel_spmd

    nc = _get_program()
    in_maps = _make_in_maps(**inputs)
    res = run_bass_kernel_spmd(
        nc, in_maps, list(range(B)), trace=trace,
        trace_cores=trace_cores,
    )
    out = np.stack([res.results[b]["outT"].T for b in range(B)]).astype(np.float32)
    return out, res


def kernel(**inputs):
    out, _ = run(inputs, trace=False)
    return out


# revision 18
# speedup vs baseline: 1.0485x; 1.0485x over previous
"""Trainium2 Bass kernel for a GNN message-passing layer.

Reference computation (per batch b):
    m   = relu(h @ W1.T + b1)
    m   = relu(m @ W2.T + b2)
    msg = relu(A @ m)
    gx  = msg @ W_ih.T + b_ih ; gh = h @ W_hh.T + b_hh   (gates r,z,n)
    r = sig(gxr+ghr); z = sig(gxz+ghz); n = tanh(gxn + r*ghn)
    out = (1-z)*n + z*h

Sharding: pure data-parallel over B (B == n_cores == 8, one batch per
NeuronCore, no collectives). Host pre-transposes per-batch tensors into
feature-major layout so the dominant tensor A streams through the PE in
its natural layout (contraction dim on partitions) with zero on-device
transposes.

Per-core device program (all feature-major, [feature(part) x nodes]):
    m1T  = relu(W1 @ hT + b1)                  4 matmuls  (f32r, N=512)
    m2n  = node-major m2 blocks                16 matmuls (f32r, N=128)
    for each 512-node chunk c:
        msgT_c = relu(sum_k m2n_k.T @ AT[k, c])  16 matmuls (f32r, N=512)
        gates: 6 matmuls in plain fp32 (precision: sigmoid/tanh preacts
        are ~+-400 here, so operand rounding must stay at fp32 level)
        GRU elementwise on DVE/ACT
"""

import numpy as np

B, N, H = 8, 2048, 128
NCHUNK = 512
NCH = N // NCHUNK  # 4
KBLK = N // 128    # 16

_CACHE = {}


def _build_program():
    import concourse.bacc as bacc
    import concourse.tile as tile
    import concourse.mybir as mybir
    from concourse.alu_op_type import AluOpType

    f32 = mybir.dt.float32
    f32r = mybir.dt.float32r
    ACT = mybir.ActivationFunctionType

    nc = bacc.Bacc("TRN2", target_bir_lowering=False, debug=False, num_devices=B)

    # ---- DRAM I/O (per-core shard, host-pre-transposed) ----
    hT_d = nc.dram_tensor("hT", [H, N], f32, kind="ExternalInput").ap()
    # A2[q, s] = one contiguous [128, 2048] slab: 4 k-blocks (t=0..3, k=4s+t)
    # of A^T columns for node-chunk q. Host packs it this way so every DMA
    # is a full-rate 1MB contiguous transfer AND chunk q's accumulation
    # completes after quarter q of the stream.
    A2_d = nc.dram_tensor("A2", [NCH, KBLK // 4, H, N], f32r, kind="ExternalInput").ap()
    w1t_d = nc.dram_tensor("W1T", [H, H], f32, kind="ExternalInput").ap()
    w2t_d = nc.dram_tensor("W2T", [H, H], f32, kind="ExternalInput").ap()
    wih_d = nc.dram_tensor("WihT", [H, 3 * H], f32, kind="ExternalInput").ap()
    whh_d = nc.dram_tensor("WhhT", [H, 3 * H], f32r, kind="ExternalInput").ap()
    b1_d = nc.dram_tensor("b1c", [H, 1], f32, kind="ExternalInput").ap()
    b2b_d = nc.dram_tensor("b2b", [H, H], f32, kind="ExternalInput").ap()
    brz_d = nc.dram_tensor("brz", [H, 2], f32, kind="ExternalInput").ap()
    bihn_d = nc.dram_tensor("bihn", [H, 1], f32, kind="ExternalInput").ap()
    bhhn_d = nc.dram_tensor("bhhn", [H, 1], f32, kind="ExternalInput").ap()
    out_d = nc.dram_tensor("outT", [H, N], f32, kind="ExternalOutput").ap()

    with tile.TileContext(nc) as tc:
        with (
            tc.tile_pool(name="consts", bufs=1) as cp,
            tc.tile_pool(name="big", bufs=1) as bp,
            tc.tile_pool(name="at", bufs=8) as ap_,
            tc.tile_pool(name="msgp", bufs=2) as mp,
            tc.tile_pool(name="tmp", bufs=2) as tp,
            tc.tile_pool(name="outp", bufs=2) as op_,
            tc.tile_pool(name="psum", bufs=1, space="PSUM") as pp,
        ):
            # ---- constant loads ----
            w1t = cp.tile([H, H], f32, tag="w1t")
            w2t = cp.tile([H, H], f32, tag="w2t")
            wih = cp.tile([H, 3 * H], f32, tag="wih")
            whh = cp.tile([H, 3 * H], f32r, tag="whh")
            b1 = cp.tile([H, 1], f32, tag="b1")
            b2b = cp.tile([H, H], f32, tag="b2b")
            brz = cp.tile([H, 2], f32, tag="brz")
            bihn = cp.tile([H, 1], f32, tag="bihn")
            bhhn = cp.tile([H, 1], f32, tag="bhhn")
            hT = bp.tile([H, N], f32, tag="hT")
            hTr = bp.tile([H, N], f32r, tag="hTr")  # f32r copy for gh matmuls
            m1T = bp.tile([H, N], f32, tag="m1T")
            m2n = bp.tile([H, N], f32r, tag="m2n")  # block k = cols [128k,128k+128)

            # constants + hT on the ACT (scalar) HWDGE ring so the sync ring
            # can start streaming A at t=0
            nc.scalar.dma_start(hT[:], hT_d[:])
            nc.scalar.dma_start(w1t[:], w1t_d[:])
            nc.scalar.dma_start(w2t[:], w2t_d[:])
            nc.scalar.dma_start(wih[:], wih_d[:])
            nc.scalar.dma_start(whh[:], whh_d[:])
            nc.scalar.dma_start(b1[:], b1_d[:])
            nc.scalar.dma_start(b2b[:], b2b_d[:])
            nc.scalar.dma_start(brz[:], brz_d[:])
            nc.scalar.dma_start(bihn[:], bihn_d[:])
            nc.scalar.dma_start(bhhn[:], bhhn_d[:])

            # ---- m1T = relu(W1 @ hT + b1), feature-major ----
            for c in range(NCH):
                sl = slice(c * NCHUNK, (c + 1) * NCHUNK)
                ps_m1 = pp.tile([H, NCHUNK], f32, tag="acc", bufs=4)
                nc.tensor.matmul(
                    ps_m1[:], w1t[:], hT[:, sl],
                    start=True, stop=True,
                )
                nc.scalar.activation(m1T[:, sl], ps_m1[:], ACT.Relu, bias=b1[:, 0:1])

            # ---- m2 node-major blocks: m2n[:, blk k] = relu(m1T_k.T @ W2T + b2) ----
            for k in range(KBLK):
                kb = slice(k * H, (k + 1) * H)
                ps_m2 = pp.tile([H, H], f32, tag="acc", bufs=4)
                nc.tensor.matmul(
                    ps_m2[:], m1T[:, kb], w2t[:],
                    start=True, stop=True,
                )
                m2pre = tp.tile([H, H], f32, tag="m2pre")
                nc.vector.tensor_add(m2pre[:], ps_m2[:], b2b[:])
                nc.scalar.activation(m2n[:, kb], m2pre[:], ACT.Relu)

            nc.scalar.copy(hTr[:], hT[:])

            # ---- stream A quarter by quarter; chunk q's msg completes after
            # quarter q, so its gates overlap quarter q+1's DMA ----
            for q in range(NCH):
                sl = slice(q * NCHUNK, (q + 1) * NCHUNK)

                ps_msg = pp.tile([H, NCHUNK], f32, tag="msg", bufs=3)
                for s in range(KBLK // 4):
                    at = ap_.tile([H, N], f32r, tag="at")
                    nc.sync.dma_start(at[:], A2_d[q, s])
                    for t in range(4):
                        k = 4 * s + t
                        nc.tensor.matmul(
                            ps_msg[:],
                            m2n[:, k * H:(k + 1) * H],
                            at[:, t * NCHUNK:(t + 1) * NCHUNK],
                            start=(k == 0), stop=(k == KBLK - 1),
                        )

                # gh gates for this chunk (depend only on h): emitted here so
                # the PE runs them in this quarter's DMA gaps
                ghr = tp.tile([H, NCHUNK], f32, tag="ghr")
                ps_ghr = pp.tile([H, NCHUNK], f32, tag="acc", bufs=4)
                nc.tensor.matmul(ps_ghr[:], whh[:, 0:H], hTr[:, sl], start=True, stop=True)
                nc.scalar.activation(ghr[:], ps_ghr[:], ACT.Identity, bias=brz[:, 0:1])
                ghz = tp.tile([H, NCHUNK], f32, tag="ghz")
                ps_ghz = pp.tile([H, NCHUNK], f32, tag="acc", bufs=4)
                nc.tensor.matmul(ps_ghz[:], whh[:, H:2 * H], hTr[:, sl], start=True, stop=True)
                nc.scalar.activation(ghz[:], ps_ghz[:], ACT.Identity, bias=brz[:, 1:2])
                ghn = tp.tile([H, NCHUNK], f32, tag="ghn")
                ps_ghn = pp.tile([H, NCHUNK], f32, tag="acc", bufs=4)
                nc.tensor.matmul(ps_ghn[:], whh[:, 2 * H:3 * H], hTr[:, sl], start=True, stop=True)
                nc.scalar.activation(ghn[:], ps_ghn[:], ACT.Identity, bias=bhhn[:, 0:1])

                msgT = mp.tile([H, NCHUNK], f32, tag="msgT")
                nc.scalar.activation(msgT[:], ps_msg[:], ACT.Relu)

                ps_gxr = pp.tile([H, NCHUNK], f32, tag="acc", bufs=4)
                nc.tensor.matmul(ps_gxr[:], wih[:, 0:H], msgT[:], start=True, stop=True)
                rpre = tp.tile([H, NCHUNK], f32, tag="rpre")
                nc.vector.tensor_add(rpre[:], ps_gxr[:], ghr[:])
                r = tp.tile([H, NCHUNK], f32, tag="r")
                nc.scalar.activation(r[:], rpre[:], ACT.Sigmoid)

                ps_gxz = pp.tile([H, NCHUNK], f32, tag="acc", bufs=4)
                nc.tensor.matmul(ps_gxz[:], wih[:, H:2 * H], msgT[:], start=True, stop=True)
                zpre = tp.tile([H, NCHUNK], f32, tag="zpre")
                nc.vector.tensor_add(zpre[:], ps_gxz[:], ghz[:])
                z = tp.tile([H, NCHUNK], f32, tag="z")
                nc.scalar.activation(z[:], zpre[:], ACT.Sigmoid)

                ps_gxn = pp.tile([H, NCHUNK], f32, tag="acc", bufs=4)
                nc.tensor.matmul(ps_gxn[:], wih[:, 2 * H:3 * H], msgT[:], start=True, stop=True)

                # t = r * (ghn + bhhn) ; npre = t + gxn ; n = tanh(npre + bihn)
                t = tp.tile([H, NCHUNK], f32, tag="t")
                nc.vector.tensor_mul(t[:], r[:], ghn[:])
                npre = tp.tile([H, NCHUNK], f32, tag="npre")
                nc.vector.tensor_add(npre[:], t[:], ps_gxn[:])
                nn = tp.tile([H, NCHUNK], f32, tag="nn")
                nc.scalar.activation(nn[:], npre[:], ACT.Tanh, bias=bihn[:, 0:1])

                # out = n + z * (h - n); chunks 0-2 on otherwise-idle GPSIMD
                # (overlaps next quarter's stream), last chunk on fast DVE
                # since it is the exposed tail
                eng = nc.vector if q == NCH - 1 else nc.gpsimd
                d = tp.tile([H, NCHUNK], f32, tag="d")
                eng.tensor_sub(d[:], hT[:, sl], nn[:])
                e = tp.tile([H, NCHUNK], f32, tag="e")
                eng.tensor_mul(e[:], z[:], d[:])
                outc = op_.tile([H, NCHUNK], f32, tag="outc")
                eng.tensor_add(outc[:], nn[:], e[:])
                nc.scalar.dma_start(out_d[:, sl], outc[:])

    nc.compile()
    return nc


def _get_program():
    if "nc" not in _CACHE:
        _CACHE["nc"] = _build_program()
    return _CACHE["nc"]


def _make_in_maps(h, A, W1, b1, W2, b2, W_ih, W_hh, b_ih, b_hh):
    f = np.float32
    shared = {
        "W1T": np.ascontiguousarray(W1.T, dtype=f),
        "W2T": np.ascontiguousarray(W2.T, dtype=f),
        "WihT": np.ascontiguousarray(W_ih.T, dtype=f),
        "WhhT": np.ascontiguousarray(W_hh.T, dtype=f),
        "b1c": np.ascontiguousarray(b1.reshape(H, 1), dtype=f),
        "b2b": np.ascontiguousarray(np.tile(b2.reshape(1, H), (H, 1)), dtype=f),
        "brz": np.ascontiguousarray(
            np.stack([(b_ih + b_hh)[0:H], (b_ih + b_hh)[H:2 * H]], axis=1), dtype=f
        ),
        "bihn": np.ascontiguousarray(b_ih[2 * H:3 * H].reshape(H, 1), dtype=f),
        "bhhn": np.ascontiguousarray(b_hh[2 * H:3 * H].reshape(H, 1), dtype=f),
    }
    in_maps = []
    for b in range(B):
        m = dict(shared)
        m["hT"] = np.ascontiguousarray(np.asarray(h[b]).T, dtype=f)
        # A2[q, s, p, t*512+j] = A^T[(4s+t)*128 + p, q*512 + j]
        AT = np.asarray(A[b]).T.astype(f)                      # [2048 m, 2048 n]
        A2 = (AT.reshape(KBLK // 4, 4, H, NCH, NCHUNK)         # [s, t, p, q, j]
                .transpose(3, 0, 2, 1, 4)                      # [q, s, p, t, j]
                .reshape(NCH, KBLK // 4, H, N))
        m["A2"] = np.ascontiguousarray(A2)
        in_maps.append(m)
    return in_maps


def run(inputs, trace=False, trace_cores=None):
    """Build (cached), run on 8 cores, return (output, BassKernelResults)."""
    from concourse.bass_utils import run_bass_kernel_spmd

    nc = _get_program()
    in_maps = _make_in_maps(**inputs)
    res = run_bass_kernel_spmd(
        nc, in_maps, list(range(B)), trace=trace,
        trace_cores=trace_cores,
    )
    out = np.stack([res.results[b]["outT"].T for b in range(B)]).astype(np.float32)
    return out, res


def kernel(**inputs):
    out, _ = run(inputs, trace=False)
    return out


# revision 19
# speedup vs baseline: 1.0682x; 1.0187x over previous
"""Trainium2 Bass kernel for a GNN message-passing layer.

Reference computation (per batch b):
    m   = relu(h @ W1.T + b1)
    m   = relu(m @ W2.T + b2)
    msg = relu(A @ m)
    gx  = msg @ W_ih.T + b_ih ; gh = h @ W_hh.T + b_hh   (gates r,z,n)
    r = sig(gxr+ghr); z = sig(gxz+ghz); n = tanh(gxn + r*ghn)
    out = (1-z)*n + z*h

Sharding: pure data-parallel over B (B == n_cores == 8, one batch per
NeuronCore, no collectives). Host pre-transposes per-batch tensors into
feature-major layout so the dominant tensor A streams through the PE in
its natural layout (contraction dim on partitions) with zero on-device
transposes.

Per-core device program (all feature-major, [feature(part) x nodes]):
    m1T  = relu(W1 @ hT + b1)                  4 matmuls  (f32r, N=512)
    m2n  = node-major m2 blocks                16 matmuls (f32r, N=128)
    for each 512-node chunk c:
        msgT_c = relu(sum_k m2n_k.T @ AT[k, c])  16 matmuls (f32r, N=512)
        gates: 6 matmuls in plain fp32 (precision: sigmoid/tanh preacts
        are ~+-400 here, so operand rounding must stay at fp32 level)
        GRU elementwise on DVE/ACT
"""

import numpy as np

B, N, H = 8, 2048, 128
NCHUNK = 512
NCH = N // NCHUNK  # 4
KBLK = N // 128    # 16

_CACHE = {}


def _build_program():
    import concourse.bacc as bacc
    import concourse.tile as tile
    import concourse.mybir as mybir
    from concourse.alu_op_type import AluOpType

    f32 = mybir.dt.float32
    f32r = mybir.dt.float32r
    ACT = mybir.ActivationFunctionType

    nc = bacc.Bacc("TRN2", target_bir_lowering=False, debug=False, num_devices=B)

    # ---- DRAM I/O (per-core shard, host-pre-transposed) ----
    hT_d = nc.dram_tensor("hT", [H, N], f32, kind="ExternalInput").ap()
    # A2[q, s] = one contiguous [128, 2048] slab: 4 k-blocks (t=0..3, k=4s+t)
    # of A^T columns for node-chunk q. Host packs it this way so every DMA
    # is a full-rate 1MB contiguous transfer AND chunk q's accumulation
    # completes after quarter q of the stream.
    A2_d = nc.dram_tensor("A2", [NCH, KBLK // 4, H, N], f32r, kind="ExternalInput").ap()
    w1t_d = nc.dram_tensor("W1T", [H, H], f32, kind="ExternalInput").ap()
    w2t_d = nc.dram_tensor("W2T", [H, H], f32, kind="ExternalInput").ap()
    wih_d = nc.dram_tensor("WihT", [H, 3 * H], f32, kind="ExternalInput").ap()
    whh_d = nc.dram_tensor("WhhT", [H, 3 * H], f32r, kind="ExternalInput").ap()
    b1_d = nc.dram_tensor("b1c", [H, 1], f32, kind="ExternalInput").ap()
    b2b_d = nc.dram_tensor("b2b", [H, H], f32, kind="ExternalInput").ap()
    brz_d = nc.dram_tensor("brz", [H, 2], f32, kind="ExternalInput").ap()
    bihn_d = nc.dram_tensor("bihn", [H, 1], f32, kind="ExternalInput").ap()
    bhhn_d = nc.dram_tensor("bhhn", [H, 1], f32, kind="ExternalInput").ap()
    out_d = nc.dram_tensor("outT", [H, N], f32, kind="ExternalOutput").ap()

    with tile.TileContext(nc) as tc:
        with (
            tc.tile_pool(name="consts", bufs=1) as cp,
            tc.tile_pool(name="big", bufs=1) as bp,
            tc.tile_pool(name="at", bufs=8) as ap_,
            tc.tile_pool(name="msgp", bufs=2) as mp,
            tc.tile_pool(name="tmp", bufs=2) as tp,
            tc.tile_pool(name="outp", bufs=2) as op_,
            tc.tile_pool(name="psum", bufs=1, space="PSUM") as pp,
        ):
            # ---- constant loads ----
            w1t = cp.tile([H, H], f32, tag="w1t")
            w2t = cp.tile([H, H], f32, tag="w2t")
            wih = cp.tile([H, 3 * H], f32, tag="wih")
            whh = cp.tile([H, 3 * H], f32r, tag="whh")
            b1 = cp.tile([H, 1], f32, tag="b1")
            b2b = cp.tile([H, H], f32, tag="b2b")
            brz = cp.tile([H, 2], f32, tag="brz")
            bihn = cp.tile([H, 1], f32, tag="bihn")
            bhhn = cp.tile([H, 1], f32, tag="bhhn")
            hT = bp.tile([H, N], f32, tag="hT")
            hTr = bp.tile([H, N], f32r, tag="hTr")  # f32r copy for gh matmuls
            m1T = bp.tile([H, N], f32, tag="m1T")
            m2n = bp.tile([H, N], f32r, tag="m2n")  # block k = cols [128k,128k+128)

            # constants + hT on the ACT (scalar) HWDGE ring so the sync ring
            # can start streaming A at t=0; w1t first so m1 can begin ASAP,
            # hT in 512-col chunks so m1 chunk 0 starts before the full load
            nc.scalar.dma_start(w1t[:], w1t_d[:])
            for c in range(NCH):
                sl = slice(c * NCHUNK, (c + 1) * NCHUNK)
                nc.scalar.dma_start(hT[:, sl], hT_d[:, sl])
            nc.scalar.dma_start(w2t[:], w2t_d[:])
            nc.scalar.dma_start(b1[:], b1_d[:])
            nc.scalar.dma_start(b2b[:], b2b_d[:])
            nc.scalar.dma_start(whh[:], whh_d[:])
            nc.scalar.dma_start(wih[:], wih_d[:])
            nc.scalar.dma_start(brz[:], brz_d[:])
            nc.scalar.dma_start(bihn[:], bihn_d[:])
            nc.scalar.dma_start(bhhn[:], bhhn_d[:])

            # ---- m1T = relu(W1 @ hT + b1), feature-major ----
            for c in range(NCH):
                sl = slice(c * NCHUNK, (c + 1) * NCHUNK)
                ps_m1 = pp.tile([H, NCHUNK], f32, tag="acc", bufs=4)
                nc.tensor.matmul(ps_m1[:], w1t[:], hT[:, sl], start=True, stop=True)
                nc.scalar.activation(m1T[:, sl], ps_m1[:], ACT.Relu, bias=b1[:, 0:1])

            # ---- m2 node-major blocks: m2n[:, blk k] = relu(m1T_k.T @ W2T + b2) ----
            for k in range(KBLK):
                kb = slice(k * H, (k + 1) * H)
                ps_m2 = pp.tile([H, H], f32, tag="acc", bufs=4)
                nc.tensor.matmul(ps_m2[:], m1T[:, kb], w2t[:], start=True, stop=True)
                m2pre = tp.tile([H, H], f32, tag="m2pre")
                nc.vector.tensor_add(m2pre[:], ps_m2[:], b2b[:])
                nc.scalar.activation(m2n[:, kb], m2pre[:], ACT.Relu)

            nc.scalar.copy(hTr[:], hT[:])

            # ---- software-pipelined stream: emit quarter q's msg matmuls,
            # then chunk q-1's gates, so the in-order PE never head-of-line
            # blocks on the ACT relu at a quarter boundary ----
            msgTs = [None] * NCH

            def emit_msg_quarter(q):
                ps_msg = pp.tile([H, NCHUNK], f32, tag="msg", bufs=3, name=f"psmsg{q}")
                for s_ in range(KBLK // 4):
                    at = ap_.tile([H, N], f32r, tag="at")
                    nc.sync.dma_start(at[:], A2_d[q, s_])
                    for t_ in range(4):
                        k = 4 * s_ + t_
                        nc.tensor.matmul(
                            ps_msg[:],
                            m2n[:, k * H:(k + 1) * H],
                            at[:, t_ * NCHUNK:(t_ + 1) * NCHUNK],
                            start=(k == 0), stop=(k == KBLK - 1),
                        )
                msgT = mp.tile([H, NCHUNK], f32, tag="msgT", name=f"msgT{q}")
                nc.scalar.activation(msgT[:], ps_msg[:], ACT.Relu)
                msgTs[q] = msgT

            def emit_gates(q):
                sl = slice(q * NCHUNK, (q + 1) * NCHUNK)
                msgT = msgTs[q]

                # gh gates (depend only on h), f32r fast path
                ghr = tp.tile([H, NCHUNK], f32, tag="ghr")
                ps_ghr = pp.tile([H, NCHUNK], f32, tag="acc", bufs=4)
                nc.tensor.matmul(ps_ghr[:], whh[:, 0:H], hTr[:, sl], start=True, stop=True)
                nc.scalar.activation(ghr[:], ps_ghr[:], ACT.Identity, bias=brz[:, 0:1])
                ghz = tp.tile([H, NCHUNK], f32, tag="ghz")
                ps_ghz = pp.tile([H, NCHUNK], f32, tag="acc", bufs=4)
                nc.tensor.matmul(ps_ghz[:], whh[:, H:2 * H], hTr[:, sl], start=True, stop=True)
                nc.scalar.activation(ghz[:], ps_ghz[:], ACT.Identity, bias=brz[:, 1:2])
                ghn = tp.tile([H, NCHUNK], f32, tag="ghn")
                ps_ghn = pp.tile([H, NCHUNK], f32, tag="acc", bufs=4)
                nc.tensor.matmul(ps_ghn[:], whh[:, 2 * H:3 * H], hTr[:, sl], start=True, stop=True)
                nc.scalar.activation(ghn[:], ps_ghn[:], ACT.Identity, bias=bhhn[:, 0:1])

                ps_gxr = pp.tile([H, NCHUNK], f32, tag="acc", bufs=4)
                nc.tensor.matmul(ps_gxr[:], wih[:, 0:H], msgT[:], start=True, stop=True)
                rpre = tp.tile([H, NCHUNK], f32, tag="rpre")
                nc.vector.tensor_add(rpre[:], ps_gxr[:], ghr[:])
                r = tp.tile([H, NCHUNK], f32, tag="r")
                nc.scalar.activation(r[:], rpre[:], ACT.Sigmoid)

                ps_gxz = pp.tile([H, NCHUNK], f32, tag="acc", bufs=4)
                nc.tensor.matmul(ps_gxz[:], wih[:, H:2 * H], msgT[:], start=True, stop=True)
                zpre = tp.tile([H, NCHUNK], f32, tag="zpre")
                nc.vector.tensor_add(zpre[:], ps_gxz[:], ghz[:])
                z = tp.tile([H, NCHUNK], f32, tag="z")
                nc.scalar.activation(z[:], zpre[:], ACT.Sigmoid)

                ps_gxn = pp.tile([H, NCHUNK], f32, tag="acc", bufs=4)
                nc.tensor.matmul(ps_gxn[:], wih[:, 2 * H:3 * H], msgT[:], start=True, stop=True)

                t = tp.tile([H, NCHUNK], f32, tag="t")
                nc.vector.tensor_mul(t[:], r[:], ghn[:])
                npre = tp.tile([H, NCHUNK], f32, tag="npre")
                nc.vector.tensor_add(npre[:], t[:], ps_gxn[:])
                nn = tp.tile([H, NCHUNK], f32, tag="nn")
                nc.scalar.activation(nn[:], npre[:], ACT.Tanh, bias=bihn[:, 0:1])

                # out = n + z * (h - n); early chunks on idle GPSIMD, last
                # chunk on fast DVE since it is the exposed tail
                eng = nc.vector if q == NCH - 1 else nc.gpsimd
                d = tp.tile([H, NCHUNK], f32, tag="d")
                eng.tensor_sub(d[:], hT[:, sl], nn[:])
                e = tp.tile([H, NCHUNK], f32, tag="e")
                eng.tensor_mul(e[:], z[:], d[:])
                outc = op_.tile([H, NCHUNK], f32, tag="outc")
                eng.tensor_add(outc[:], nn[:], e[:])
                nc.scalar.dma_start(out_d[:, sl], outc[:])

            for q in range(NCH):
                emit_msg_quarter(q)
                if q >= 1:
                    emit_gates(q - 1)
            emit_gates(NCH - 1)

    nc.compile()
    return nc


def _get_program():
    if "nc" not in _CACHE:
        _CACHE["nc"] = _build_program()
    return _CACHE["nc"]


def _make_in_maps(h, A, W1, b1, W2, b2, W_ih, W_hh, b_ih, b_hh):
    f = np.float32
    shared = {
        "W1T": np.ascontiguousarray(W1.T, dtype=f),
        "W2T": np.ascontiguousarray(W2.T, dtype=f),
        "WihT": np.ascontiguousarray(W_ih.T, dtype=f),
        "WhhT": np.ascontiguousarray(W_hh.T, dtype=f),
        "b1c": np.ascontiguousarray(b1.reshape(H, 1), dtype=f),
        "b2b": np.ascontiguousarray(np.tile(b2.reshape(1, H), (H, 1)), dtype=f),
        "brz": np.ascontiguousarray(
            np.stack([(b_ih + b_hh)[0:H], (b_ih + b_hh)[H:2 * H]], axis=1), dtype=f
        ),
        "bihn": np.ascontiguousarray(b_ih[2 * H:3 * H].reshape(H, 1), dtype=f),
        "bhhn": np.ascontiguousarray(b_hh[2 * H:3 * H].reshape(H, 1), dtype=f),
    }
    in_maps = []
    for b in range(B):
        m = dict(shared)
        m["hT"] = np.ascontiguousarray(np.asarray(h[b]).T, dtype=f)
        # A2[q, s, p, t*512+j] = A^T[(4s+t)*128 + p, q*512 + j]
        AT = np.asarray(A[b]).T.astype(f)                      # [2048 m, 2048 n]
        A2 = (AT.reshape(KBLK // 4, 4, H, NCH, NCHUNK)         # [s, t, p, q, j]
                .transpose(3, 0, 2, 1, 4)                      # [q, s, p, t, j]
                .reshape(NCH, KBLK // 4, H, N))
        m["A2"] = np.ascontiguousarray(A2)
        in_maps.append(m)
    return in_maps


def run(inputs, trace=False, trace_cores=None):
    """Build (cached), run on 8 cores, return (output, BassKernelResults)."""
    from concourse.bass_utils import run_bass_kernel_spmd

    nc = _get_program()
    in_maps = _make_in_maps(**inputs)
    res = run_bass_kernel_spmd(
        nc, in_maps, list(range(B)), trace=trace,
        trace_cores=trace_cores,
    )
    out = np.stack([res.results[b]["outT"].T for b in range(B)]).astype(np.float32)
    return out, res


def kernel(**inputs):
    out, _ = run(inputs, trace=False)
    return out
